# revision 18
# baseline (speedup 1.0000x reference)
"""AgentAttention Trainium2 kernel (v2).

Full inputs -> shard batch over 8 NeuronCores (2 samples each) -> Bass/Tile
kernel per core -> gather full output.

v2 redesign vs baseline:
- All PE operands bf16 (moving-operand fast path), fp16 output DMA.
- Position biases fully precomputed on host ([N,512] stage-1 and [512,N]
  stage-2 tables) and injected per tile via a single identity matmul that
  initializes the score PSUM -- replaces the 2-matmul Phi-basis rebuild.
- V is projected twice: dim-major (for dwc) and token-major directly
  (lhsT=X trick), eliminating all per-tile PE transposes of V.
- Stage-1 agent_v computed in [dim, agent] orientation with a ones column
  appended to the vtok lhsT, so the softmax denominators (column sums)
  drop out of the same matmuls; normalization folds into the WAV
  (Wproj @ agent_v) evacuation as a per-partition ACT scale.
- Stage-2 runs agent-major ([ha, n] scores via lhsT=agent-blockdiag,
  rhs=Q), eliminating the 4-per-tile PE transposes of the prob matrix.
  Softmax denominators come from indicator matmuls ([8,512] PSUM), a fast
  approximate reciprocal, and a k=8 broadcast matmul re-expands them.
- Output matmul consumes [ha, n] probs + dwc result; bproj (+ Wproj@bdwc)
  is added during the output evacuation as a per-partition DVE scalar.
- 3x3 depthwise conv stays as PE diagonal matmuls with flat shifts; the
  x-wrap columns are recomputed exactly by DVE strided ops.
"""
import sys
import numpy as np
from contextlib import ExitStack

sys.path.insert(0, "/opt/trn_rl_repo")

import ml_dtypes
import concourse.bass as bass
import concourse.tile as tile
from concourse import mybir
from concourse.bass_utils import run_bass_kernel_spmd

B, DIM, HEADS, AGENT = 16, 256, 8, 49
H = W = 64
N = H * W
HD = DIM // HEADS             # 32
SCALE = HD ** -0.5
N_CORES = 8
SPC = B // N_CORES            # 2 samples per core
NT = N // 128                 # 32 token tiles
NCH = N // 512                # 8 token chunks
AGP = 64                      # padded agent dim per head
HAP = HEADS * AGP             # 512
NPAIR = 4
PAD_BIAS = -30.0

F32 = mybir.dt.float32
F16 = mybir.dt.float16
BF16 = mybir.dt.bfloat16
AX = mybir.AxisListType
AF = mybir.ActivationFunctionType


def _pool_bins(size, out):
    return [((i * size) // out, -((-(i + 1) * size) // out)) for i in range(out)]


def _resize_matrix(n_in, n_out):
    """Row-normalized half-pixel triangle-kernel resize matrix: matches
    jax.image.resize(method='bilinear') upsampling."""
    R = np.zeros((n_out, n_in), np.float64)
    scale = n_in / n_out
    for o in range(n_out):
        c = (o + 0.5) * scale - 0.5
        for i in range(n_in):
            R[o, i] = max(0.0, 1.0 - abs(c - i))
        s = R[o].sum()
        if s > 0:
            R[o] /= s
    return R.astype(np.float32)


def _wt_layout(WT):
    """[256, 256] (cin, cout) -> [128, 2, 256] tile layout, [p, k, m]."""
    return np.ascontiguousarray(WT.reshape(2, 128, DIM).transpose(1, 0, 2))


def build_host_constants(Wq, Wkv, Wproj, bproj, Wdwc, bdwc,
                         an_bias, na_bias, ah_bias, aw_bias, ha_bias, wa_bias):
    """Pure parameter re-layout / folding on host."""
    c = {}
    Wk, Wv = Wkv[:DIM], Wkv[DIM:]
    c["WqT"] = _wt_layout(Wq.T)
    c["WkT"] = _wt_layout(Wk.T)
    c["WvT"] = _wt_layout(Wv.T)
    c["WprojT"] = _wt_layout(Wproj.T)
    # pair-major Wproj^T rows: [64, 4, 256], pair p rows = dims 64p..64p+64
    c["WprojP"] = np.ascontiguousarray(
        Wproj.T.reshape(4, 64, DIM).transpose(1, 0, 2))

    Rh = _resize_matrix(7, H)          # [64, 7]
    Rw = _resize_matrix(7, W)
    # stage-1 bias: [n, (h, a64)]
    pb1 = np.einsum('yp,xq,hapq->hayx', Rh, Rw, an_bias)   # [h, a, y, x]
    b1 = pb1 + ah_bias[0][:, :, :, 0][:, :, :, None] \
        + aw_bias[0][:, :, 0, :][:, :, None, :]            # [h, a, y, x]
    bias1 = np.full((HEADS, AGP, N), PAD_BIAS, np.float32)
    bias1[:, :AGENT, :] = b1.reshape(HEADS, AGENT, N)
    bias1 = bias1.reshape(HAP, N).T                        # [n, 512]
    c["bias1"] = np.ascontiguousarray(
        bias1.reshape(NT // 2, 2, 128, HAP).transpose(0, 2, 1, 3))

    ab1 = np.einsum('yp,xq,hapq->hayx', Rh, Rw, na_bias)   # [h, a, y, x]
    # ha_bias [1,h,64,1,a] -> [h, a, y, 1]; wa_bias [1,h,1,64,a] -> [h, a, 1, x]
    b2 = ab1 + ha_bias[0, :, :, 0, :].transpose(0, 2, 1)[:, :, :, None] \
        + wa_bias[0, :, 0, :, :].transpose(0, 2, 1)[:, :, None, :]
    biasT2 = np.full((HEADS, AGP, N), PAD_BIAS, np.float32)
    biasT2[:, :AGENT, :] = b2.reshape(HEADS, AGENT, N)
    # [chunk, 128 (row within slice), pair-slice, 512]
    c["biasT2"] = np.ascontiguousarray(
        biasT2.reshape(HAP, N).reshape(NPAIR, 128, NCH, 512)
        .transpose(2, 1, 0, 3))

    binsH, binsW = _pool_bins(H, 7), _pool_bins(W, 7)
    s49 = np.zeros((1, AGENT), np.float32)
    for p in range(7):
        for q in range(7):
            lp = binsH[p][1] - binsH[p][0]
            lq = binsW[q][1] - binsW[q][0]
            s49[0, p * 7 + q] = 1.0 / (lp * lq)
    c["S49"] = s49

    w9 = Wdwc[:, 0, :, :].reshape(DIM, 9)
    diag = np.zeros((18, 128, 128), np.float32)
    for t in range(9):
        for ct_ in range(2):
            np.fill_diagonal(diag[ct_ * 9 + t], w9[ct_ * 128:(ct_ + 1) * 128, t])
    c["DIAG"] = np.ascontiguousarray(
        diag.reshape(2, 9, 128, 128).transpose(2, 0, 1, 3))  # [128, 2, 9, 128]
    c["W9"] = np.ascontiguousarray(
        w9.reshape(2, 128, 9).transpose(1, 0, 2))            # [128, 2, 9]

    bpe = (bproj + Wproj @ bdwc).astype(np.float32)
    c["bproj2"] = np.ascontiguousarray(bpe.reshape(2, 128).T)  # [128, 2]

    ind8 = np.zeros((128, NPAIR, 8), np.float32)
    for p in range(NPAIR):
        ind8[:64, p, 2 * p] = 1.0
        ind8[64:, p, 2 * p + 1] = 1.0
    c["ind8"] = ind8
    indp = np.zeros((8, NPAIR, 128), np.float32)
    for p in range(NPAIR):
        indp[2 * p, p, :64] = 1.0
        indp[2 * p + 1, p, 64:] = 1.0
    c["indp"] = indp
    c["ident"] = np.eye(128, dtype=np.float32)
    c["ones65"] = np.ones((65, 1), np.float32)
    c["zero1"] = np.zeros((1, 128), np.float32)
    c["ones1"] = np.ones((1, 128), np.float32)
    return c


CONST_SPECS = [
    ("WqT", BF16, [128, 2, DIM]),
    ("WkT", BF16, [128, 2, DIM]),
    ("WvT", BF16, [128, 2, DIM]),
    ("WprojT", BF16, [128, 2, DIM]),
    ("WprojP", BF16, [64, 4, DIM]),
    ("DIAG", BF16, [128, 2, 9, 128]),
    ("W9", F32, [128, 2, 9]),
    ("S49", F32, [1, AGENT]),
    ("bproj2", F32, [128, 2]),
    ("ind8", BF16, [128, NPAIR, 8]),
    ("ident", BF16, [128, 128]),
    ("ones65", F32, [65, 1]),
    ("zero1", BF16, [1, 128]),
    ("ones1", BF16, [1, 128]),
]
STREAM_SPECS = [
    ("bias1", BF16, [NT // 2, 128, 2, HAP]),
    ("biasT2", BF16, [NCH, 128, NPAIR, 512]),
]


def split_multiwaits(nc, max_waits=1):
    """Walrus codegen has one sync-wait slot per instruction; split extras
    into standalone EventSemaphore waits on the same engine."""
    n_split = 0
    for f in nc.m.functions:
        for bb in f.blocks:
            new_insts = []
            changed = False
            for inst in bb.instructions:
                si = inst.sync_info
                if (si is not None and si.on_wait is not None
                        and len(si.on_wait) > max_waits and inst.is_executable()):
                    waits = list(si.on_wait)
                    extra, keep = waits[:-max_waits], waits[-max_waits:]
                    for w in extra:
                        ev = mybir.InstEventSemaphore(
                            name=f"{inst.name}-ws{n_split}",
                            engine=inst.engine, ins=[], outs=[],
                            sync_info=mybir.SyncInfo(on_wait=[w], on_update=[]),
                        )
                        new_insts.append(ev)
                        n_split += 1
                    inst.sync_info = mybir.SyncInfo(
                        on_wait=keep, on_update=list(si.on_update))
                    changed = True
                new_insts.append(inst)
            if changed:
                bb.instructions = new_insts
    return n_split


def build_nc():
    nc = bass.Bass()
    x_in = nc.dram_tensor("x", [SPC, DIM, N], BF16, kind="ExternalInput")
    out_d = nc.dram_tensor("out", [SPC, DIM, N], F16, kind="ExternalOutput")
    cst = {name: nc.dram_tensor(name, shape, dt, kind="ExternalInput")
           for name, dt, shape in CONST_SPECS + STREAM_SPECS}

    with tile.TileContext(nc) as tc, ExitStack() as ctx:
        kernel_body(ctx, tc, nc, x_in, out_d, cst)
    split_multiwaits(nc)
    return nc


def kernel_body(ctx, tc, nc, x_in, out_d, cst):
    const = ctx.enter_context(tc.tile_pool(name="const", bufs=1))
    big = ctx.enter_context(tc.tile_pool(name="big", bufs=1))
    work = ctx.enter_context(tc.tile_pool(name="work", bufs=3))
    small = ctx.enter_context(tc.tile_pool(name="small", bufs=2))
    outp = ctx.enter_context(tc.tile_pool(name="outp", bufs=3))
    # PSUM budget (8 banks): ps_main x3, ps_out x3, ps_s2 x1, ps_av x1
    dram = ctx.enter_context(tc.tile_pool(name="dram", bufs=2, space="DRAM"))
    psA = ctx.enter_context(tc.tile_pool(name="psA", bufs=3, space="PSUM"))
    psB = ctx.enter_context(tc.tile_pool(name="psB", bufs=2, space="PSUM"))
    psAcc = ctx.enter_context(tc.tile_pool(name="psAcc", bufs=1, space="PSUM"))

    ct = {}
    for name, dt, shape in CONST_SPECS:
        t = const.tile(shape, dt, name=f"c_{name}")
        nc.sync.dma_start(out=t, in_=cst[name].ap())
        ct[name] = t
    s49rep = const.tile([128, AGENT], F32, name="s49rep")
    nc.sync.dma_start(
        out=s49rep,
        in_=bass.AP(tensor=cst["S49"], offset=0, ap=[[0, 128], [1, AGENT]]))
    ct["s49rep"] = s49rep
    ct["bias1_d"] = cst["bias1"]
    ct["biasT2_d"] = cst["biasT2"]

    # persistent zero-padded agent_v blockdiag lhsT [64, 4, 128]
    avbd = const.tile([64, NPAIR, 128], BF16, name="avbd")
    nc.vector.memset(avbd, 0.0)
    ct["avbd"] = avbd
    # vtok5: [128, NT, 4, 65] token-major V with per-pair ones column
    vtok5 = big.tile([128, NT, NPAIR, 65], BF16, name="vtok5", tag="vtok5")
    nc.vector.memset(
        bass.AP(tensor=vtok5.tensor, offset=vtok5.offset + 64,
                ap=[vtok5.ap[0], [NPAIR * 65, NT], [65, NPAIR], [1, 1]]), 1.0)
    ct["vtok5"] = vtok5

    def load_x(s):
        row = []
        for hf in range(2):
            t = big.tile([128, N], BF16, name=f"x{s}{hf}", tag=f"xh{hf}")
            nc.sync.dma_start(
                out=t, in_=x_in.ap()[s, hf * 128:(hf + 1) * 128, :])
            row.append(t)
        return row

    xh = load_x(0)
    for s in range(SPC):
        xh = sample(nc, ct, s, xh, load_x, x_in, out_d, big, work, small,
                    outp, psA, psB, psAcc, dram)


def sample(nc, ct, s, xh, load_x, x_in, out_d, big, work, small, outp,
           psA, psB, psAcc, dram):
    F = 512
    vtok5 = ct["vtok5"]

    # ---- QKV projections (dim-major) ------------------------------------
    QT, KT, VT = [], [], []
    for hf in range(2):
        QT.append(big.tile([128, N], BF16, name=f"qt{hf}", tag=f"qt{hf}"))
        KT.append(big.tile([128, N], BF16, name=f"kt{hf}", tag=f"kt{hf}"))
        VT.append(big.tile([128, N], BF16, name=f"v{hf}", tag=f"v{hf}"))

    for wname, dest in (("WvT", "v"), ("WkT", "k"), ("WqT", "q")):
        wt = ct[wname]
        for mt in range(2):
            for chn in range(NCH):
                ps = psA.tile([128, F], F32, name="ps_proj", tag="ps_main")
                for kt_ in range(2):
                    nc.tensor.matmul(
                        ps,
                        lhsT=wt[:, kt_, mt * 128:(mt + 1) * 128],
                        rhs=xh[kt_][:, chn * F:(chn + 1) * F],
                        start=(kt_ == 0), stop=(kt_ == 1))
                if dest == "q":
                    nc.vector.tensor_copy(
                        out=QT[mt][:, chn * F:(chn + 1) * F], in_=ps)
                elif dest == "k":
                    nc.vector.tensor_copy(
                        out=KT[mt][:, chn * F:(chn + 1) * F], in_=ps)
                else:
                    nc.scalar.copy(out=VT[mt][:, chn * F:(chn + 1) * F], in_=ps)

    # ---- token-major V (direct projection, no transposes) ---------------
    for t in range(NT):
        ps = psB.tile([128, DIM], F32, name="ps_vt", tag="ps_out", bufs=3)
        for kt_ in range(2):
            nc.tensor.matmul(
                ps, lhsT=xh[kt_][:, t * 128:(t + 1) * 128],
                rhs=ct["WvT"][:, kt_, :],
                start=(kt_ == 0), stop=(kt_ == 1))
        nc.vector.tensor_copy(
            out=bass.AP(tensor=vtok5.tensor,
                        offset=vtok5.offset + t * NPAIR * 65,
                        ap=[vtok5.ap[0], [65, NPAIR], [1, 64]]),
            in_=ps.rearrange("p (a b) -> p a b", a=NPAIR))

    # ---- agent tokens: pool X -> project -> blockdiag (gpsimd pooling) --
    binsH, binsW = _pool_bins(H, 7), _pool_bins(W, 7)
    XpH = []
    for hf in range(2):
        x3 = xh[hf].rearrange("p (y x) -> p y x", y=H)
        qx = small.tile([128, H, 7], F32, name="qx", tag="qx")
        for q, (s0, e0) in enumerate(binsW):
            nc.vector.tensor_reduce(
                out=qx[:, :, q:q + 1], in_=x3[:, :, s0:e0],
                axis=AX.X, op=mybir.AluOpType.add)
        xp = small.tile([128, 7, 7], F32, name="xp", tag="xp")
        qxf = qx.rearrange("p y q -> p (y q)")
        for p, (s0, e0) in enumerate(binsH):
            seg = bass.AP(tensor=qxf.tensor, offset=qxf.offset + s0 * 7,
                          ap=[qxf.ap[0], [1, 7], [7, e0 - s0]])
            nc.vector.tensor_reduce(
                out=xp[:, p, :], in_=seg, axis=AX.X, op=mybir.AluOpType.add)
        xpb = small.tile([128, AGENT], BF16, name="xpb", tag="xpb")
        nc.vector.tensor_mul(
            out=xpb, in0=xp.rearrange("p a b -> p (a b)"), in1=ct["s49rep"])
        XpH.append(xpb)

    agentT = []
    for mt in range(2):
        ps = psB.tile([128, AGENT], F32, name="ps_ag", tag="ps_out", bufs=3)
        for kt_ in range(2):
            nc.tensor.matmul(
                ps,
                lhsT=ct["WqT"][:, kt_, mt * 128:(mt + 1) * 128],
                rhs=XpH[kt_], start=(kt_ == 0), stop=(kt_ == 1))
        at = small.tile([128, AGENT], BF16, name=f"at{mt}", tag=f"at{mt}")
        nc.scalar.activation(out=at, in_=ps, func=AF.Copy, scale=SCALE)
        agentT.append(at)

    bd = []
    for hf in range(2):
        b = small.tile([128, 4 * AGP], BF16, name=f"bd{hf}", tag=f"bd{hf}")
        nc.vector.memset(b, 0.0)
        for hl in range(4):
            nc.vector.tensor_copy(
                out=b[hl * 32:(hl + 1) * 32, hl * AGP:hl * AGP + AGENT],
                in_=agentT[hf][hl * 32:(hl + 1) * 32, :])
        bd.append(b)

    # prefetch next sample's input; same tags, so the DMA waits for this
    # sample's last xh reader and overlaps the rest of this sample
    xh_next = load_x(s + 1) if s + 1 < SPC else None

    # ---- dwc: diagonal matmuls, x-wrap columns fixed on DVE -------------
    DWall = big.tile([128, 2, N], BF16, name="dwall", tag="dwall")
    TAPS = [(0, 0)] + [(dy, dx) for dy in (-1, 0, 1) for dx in (-1, 0, 1)
                       if (dy, dx) != (0, 0)]
    for cti in range(2):
        v = VT[cti]
        for chn in range(NCH):
            ps = psA.tile([128, F], F32, name="ps_dw", tag="ps_main")
            lo = chn * F
            for k, (dy, dx) in enumerate(TAPS):
                t9 = (dy + 1) * 3 + (dx + 1)
                d = dy * W + dx
                a = max(0, -(lo + d))
                b_ = max(0, (lo + F + d) - N)
                nc.tensor.matmul(
                    ps[:, a:F - b_], lhsT=ct["DIAG"][:, cti, t9, :],
                    rhs=v[:, lo + d + a:lo + F + d - b_],
                    start=(k == 0), stop=(k == 8), skip_group_check=True)
            nc.scalar.copy(out=DWall[:, cti, lo:lo + F], in_=ps)
        # x-boundary columns: recompute exactly with strided DVE ops
        dwp = DWall[:, cti, :]
        for xb, dxs in ((0, (0, 1)), (W - 1, (-1, 0))):
            first = True
            for dy in (0, -1, 1):     # dy=0 first: full row range overwrite
                for dx in dxs:
                    t9 = (dy + 1) * 3 + (dx + 1)
                    rs, re = max(0, -dy), H - max(0, dy)
                    nr = re - rs
                    o_ap = bass.AP(tensor=dwp.tensor,
                                   offset=dwp.offset + rs * W + xb,
                                   ap=[dwp.ap[0], [W, nr]])
                    v_ap = bass.AP(tensor=v.tensor,
                                   offset=v.offset + (rs + dy) * W + xb + dx,
                                   ap=[v.ap[0], [W, nr]])
                    wcol = ct["W9"][:, cti, t9:t9 + 1]
                    if first:
                        nc.vector.tensor_scalar_mul(
                            out=o_ap, in0=v_ap, scalar1=wcol)
                        first = False
                    else:
                        nc.vector.scalar_tensor_tensor(
                            out=o_ap, in0=v_ap, scalar=wcol, in1=o_ap,
                            op0=mybir.AluOpType.mult, op1=mybir.AluOpType.add)

    # ---- stage 1: agent -> kv attention (token-major) -------------------
    ps_av = psAcc.tile([65, 512], F32, name="ps_av", tag="ps_av")
    # clear has_written bits for the whole bank
    nc.tensor.matmul(ps_av[0:65, 0:128], lhsT=ct["zero1"][:, 0:65],
                     rhs=ct["ones1"], start=True, stop=False,
                     skip_group_check=True)
    # av8 matmuls are skewed one tile behind the score/exp pipeline so the
    # PE never waits on the exp of the tile it just scored.
    e1s = {}
    btps = {}
    for t in range(NT + 1):
        if t < NT:
            if t % 2 == 0:
                btp = work.tile([128, 2, HAP], BF16, name="bt", tag="bt")
                nc.sync.dma_start(out=btp, in_=ct["bias1_d"].ap()[t // 2])
                btps[t // 2] = btp
            btp = btps[t // 2]
            ps = psA.tile([128, HAP], F32, name="ps_s1", tag="ps_main")
            nc.tensor.matmul(ps, lhsT=ct["ident"], rhs=btp[:, t % 2, :],
                             start=True, stop=False, skip_group_check=True)
            for hf in range(2):
                nc.tensor.matmul(
                    ps[:, hf * 256:(hf + 1) * 256],
                    lhsT=KT[hf][:, t * 128:(t + 1) * 128],
                    rhs=bd[hf], start=False, stop=(hf == 1),
                    skip_group_check=True)
            e1 = work.tile([128, HAP], BF16, name="e1", tag="e1")
            nc.scalar.activation(out=e1, in_=ps, func=AF.Exp)
            e1s[t] = e1
        if t >= 1:
            e1 = e1s.pop(t - 1)
            for p in range(NPAIR):
                nc.tensor.matmul(
                    ps_av[0:65, p * 128:(p + 1) * 128],
                    lhsT=vtok5[:, t - 1, p, :],
                    rhs=e1[:, p * 128:(p + 1) * 128],
                    start=False, stop=(t - 1 == NT - 1),
                    skip_group_check=True)

    # ---- stage 2 (agent-major) + fused output, 2-chunk software pipeline
    a2n = big.tile([128, NPAIR, N], BF16, name="a2n", tag="a2n")
    WAV = small.tile([128, NPAIR, DIM], BF16, name="wav", tag="wav")

    def s1_post():
        # colsum row -> SBUF, transpose per pair into [128,4], reciprocal
        cs1 = small.tile([65, HAP], F32, name="cs1", tag="cs1")
        nc.scalar.copy(out=cs1[64:65, :], in_=ps_av[64:65, :])
        ps4 = psB.tile([128, NPAIR], F32, name="ps_cs4", tag="ps_s2",
                       bufs=1)
        for p in range(NPAIR):
            nc.tensor.transpose(
                ps4[:, p:p + 1], in_=cs1[64:65, p * 128:(p + 1) * 128],
                identity=ct["ones65"][64:65, :])
        rcp4 = small.tile([128, NPAIR], F32, name="rcp4", tag="rcp4")
        nc.vector.reciprocal(out=rcp4, in_=ps4)
        avbd = ct["avbd"]
        for p in range(NPAIR):
            nc.scalar.copy(
                out=avbd[0:32, p, 0:AGENT],
                in_=ps_av[0:32, p * 128:p * 128 + AGENT])
            nc.scalar.copy(
                out=avbd[32:64, p, 64:64 + AGENT],
                in_=ps_av[32:64, p * 128 + 64:p * 128 + 64 + AGENT])
        for p in range(NPAIR):
            psW = psB.tile([128, DIM], F32, name="ps_wav", tag="ps_out", bufs=3)
            nc.tensor.matmul(psW, lhsT=avbd[:, p, :],
                             rhs=ct["WprojP"][:, p, :], start=True, stop=True)
            nc.scalar.activation(out=WAV[:, p, :], in_=psW, func=AF.Copy,
                                 scale=rcp4[:, p:p + 1])

    def s2_scores(c):
        bt2 = work.tile([128, NPAIR, F], BF16, name="btT", tag="bt")
        nc.sync.dma_start(out=bt2, in_=ct["biasT2_d"].ap()[c])
        for p in range(NPAIR):
            ps = psA.tile([128, F], F32, name="ps_s2", tag="ps_main")
            nc.tensor.matmul(ps, lhsT=ct["ident"], rhs=bt2[:, p, :],
                             start=True, stop=False, skip_group_check=True)
            nc.tensor.matmul(
                ps, lhsT=bd[p // 2][:, (p % 2) * 128:(p % 2) * 128 + 128],
                rhs=QT[p // 2][:, c * F:(c + 1) * F],
                start=False, stop=True, skip_group_check=True)
            nc.scalar.activation(out=a2n[:, p, c * F:(c + 1) * F], in_=ps,
                                 func=AF.Exp)
        ps_s2 = psB.tile([8, F], F32, name="ps_den", tag="ps_s2", bufs=1)
        for p in range(NPAIR):
            nc.tensor.matmul(
                ps_s2, lhsT=ct["ind8"][:, p, :],
                rhs=a2n[:, p, c * F:(c + 1) * F],
                start=(p == 0), stop=(p == NPAIR - 1))
        # r2 = 1/s2 via exp(-ln(s2)) on the scalar engine (s2 in [46, 53])
        r2f = small.tile([8, F], F32, name="r2f", tag="r2f")
        nc.scalar.activation(out=r2f, in_=ps_s2, func=AF.Ln)
        r2c = small.tile([8, F], BF16, name="r2c", tag="r2c")
        nc.scalar.activation(out=r2c, in_=r2f, func=AF.Exp, scale=-1.0)
        r2d = dram.tile([8, F], BF16, name="r2d", tag="r2d")
        nc.sync.dma_start(out=r2d, in_=r2c)
        return r2d

    def s2_norm(c, r2d):
        r2rep = work.tile([128, NPAIR, F], BF16, name="r2rep", tag="r2rep")
        nc.sync.dma_start(
            out=r2rep[0:64, :, :],
            in_=bass.AP(tensor=r2d.tensor, offset=r2d.offset,
                        ap=[[0, 64], [2 * F, NPAIR], [1, F]]))
        nc.sync.dma_start(
            out=r2rep[64:128, :, :],
            in_=bass.AP(tensor=r2d.tensor, offset=r2d.offset + F,
                        ap=[[0, 64], [2 * F, NPAIR], [1, F]]))
        for p in range(NPAIR):
            sl = a2n[:, p, c * F:(c + 1) * F]
            nc.vector.tensor_mul(out=sl, in0=sl, in1=r2rep[:, p, :])

    def s2_out(c):
        for ot in range(2):
            ps_o = psB.tile([128, F], F32, name="ps_o", tag="ps_out", bufs=3)
            for p in range(NPAIR):
                nc.tensor.matmul(
                    ps_o, lhsT=WAV[:, p, ot * 128:(ot + 1) * 128],
                    rhs=a2n[:, p, c * F:(c + 1) * F],
                    start=(p == 0), stop=False, skip_group_check=True)
            for kt_ in range(2):
                nc.tensor.matmul(
                    ps_o,
                    lhsT=ct["WprojT"][:, kt_, ot * 128:(ot + 1) * 128],
                    rhs=DWall[:, kt_, c * F:(c + 1) * F],
                    start=False, stop=(kt_ == 1), skip_group_check=True)
            o = outp.tile([128, F], F16, name="o_st", tag="o_st")
            nc.vector.tensor_scalar_add(
                out=o, in0=ps_o, scalar1=ct["bproj2"][:, ot:ot + 1])
            nc.gpsimd.dma_start(
                out=out_d.ap()[s, ot * 128:(ot + 1) * 128,
                               c * F:(c + 1) * F],
                in_=o)

    r2cs = {}
    for c in range(NCH + 2):
        if c < NCH:
            r2cs[c] = s2_scores(c)
        if c == 0:
            s1_post()
        if 1 <= c <= NCH:
            s2_norm(c - 1, r2cs.pop(c - 1))
        if c >= 2:
            s2_out(c - 2)
    return xh_next


def kernel(**inputs):
    x = np.asarray(inputs["x"], np.float32)
    host = build_host_constants(
        np.asarray(inputs["Wq"], np.float32),
        np.asarray(inputs["Wkv"], np.float32),
        np.asarray(inputs["Wproj"], np.float32),
        np.asarray(inputs["bproj"], np.float32),
        np.asarray(inputs["Wdwc"], np.float32),
        np.asarray(inputs["bdwc"], np.float32),
        np.asarray(inputs["an_bias"], np.float32),
        np.asarray(inputs["na_bias"], np.float32),
        np.asarray(inputs["ah_bias"], np.float32),
        np.asarray(inputs["aw_bias"], np.float32),
        np.asarray(inputs["ha_bias"], np.float32),
        np.asarray(inputs["wa_bias"], np.float32),
    )
    nc = build_nc()

    specs = CONST_SPECS + STREAM_SPECS
    np_dt = {mybir.dt.float32: np.float32, mybir.dt.float16: np.float16,
             mybir.dt.bfloat16: ml_dtypes.bfloat16}

    const_map = {name: np.asarray(host[name], np_dt[dt])
                 for name, dt, _ in specs}
    xs = x.reshape(B, DIM, N)
    in_maps = []
    for c in range(N_CORES):
        m = dict(const_map)
        m["x"] = np.asarray(xs[c * SPC:(c + 1) * SPC], ml_dtypes.bfloat16)
        in_maps.append(m)

    res = run_bass_kernel_spmd(nc, in_maps, core_ids=list(range(N_CORES)))
    out = np.concatenate([res.results[c]["out"] for c in range(N_CORES)],
                         axis=0)
    return out.astype(np.float32).reshape(B, DIM, H, W)


# revision 19
# speedup vs baseline: 1.1191x; 1.1191x over previous
"""AgentAttention Trainium2 kernel (v2).

Full inputs -> shard batch over 8 NeuronCores (2 samples each) -> Bass/Tile
kernel per core -> gather full output.

v2 redesign vs baseline:
- All PE operands bf16 (moving-operand fast path), fp16 output DMA.
- Position biases fully precomputed on host ([N,512] stage-1 and [512,N]
  stage-2 tables) and injected per tile via a single identity matmul that
  initializes the score PSUM -- replaces the 2-matmul Phi-basis rebuild.
- V is projected twice: dim-major (for dwc) and token-major directly
  (lhsT=X trick), eliminating all per-tile PE transposes of V.
- Stage-1 agent_v computed in [dim, agent] orientation with a ones column
  appended to the vtok lhsT, so the softmax denominators (column sums)
  drop out of the same matmuls; normalization folds into the WAV
  (Wproj @ agent_v) evacuation as a per-partition ACT scale.
- Stage-2 runs agent-major ([ha, n] scores via lhsT=agent-blockdiag,
  rhs=Q), eliminating the 4-per-tile PE transposes of the prob matrix.
  Softmax denominators come from indicator matmuls ([8,512] PSUM), a fast
  approximate reciprocal, and a k=8 broadcast matmul re-expands them.
- Output matmul consumes [ha, n] probs + dwc result; bproj (+ Wproj@bdwc)
  is added during the output evacuation as a per-partition DVE scalar.
- 3x3 depthwise conv stays as PE diagonal matmuls with flat shifts; the
  x-wrap columns are recomputed exactly by DVE strided ops.
"""
import sys
import numpy as np
from contextlib import ExitStack

sys.path.insert(0, "/opt/trn_rl_repo")

import ml_dtypes
import concourse.bass as bass
import concourse.tile as tile
from concourse import mybir
from concourse.bass_utils import run_bass_kernel_spmd

B, DIM, HEADS, AGENT = 16, 256, 8, 49
H = W = 64
N = H * W
HD = DIM // HEADS             # 32
SCALE = HD ** -0.5
N_CORES = 8
SPC = B // N_CORES            # 2 samples per core
NT = N // 128                 # 32 token tiles
NCH = N // 512                # 8 token chunks
AGP = 64                      # padded agent dim per head
HAP = HEADS * AGP             # 512
NPAIR = 4
PAD_BIAS = -30.0

F32 = mybir.dt.float32
F16 = mybir.dt.float16
BF16 = mybir.dt.bfloat16
AX = mybir.AxisListType
AF = mybir.ActivationFunctionType


def _pool_bins(size, out):
    return [((i * size) // out, -((-(i + 1) * size) // out)) for i in range(out)]


def _resize_matrix(n_in, n_out):
    """Row-normalized half-pixel triangle-kernel resize matrix: matches
    jax.image.resize(method='bilinear') upsampling."""
    R = np.zeros((n_out, n_in), np.float64)
    scale = n_in / n_out
    for o in range(n_out):
        c = (o + 0.5) * scale - 0.5
        for i in range(n_in):
            R[o, i] = max(0.0, 1.0 - abs(c - i))
        s = R[o].sum()
        if s > 0:
            R[o] /= s
    return R.astype(np.float32)


def _wt_layout(WT):
    """[256, 256] (cin, cout) -> [128, 2, 256] tile layout, [p, k, m]."""
    return np.ascontiguousarray(WT.reshape(2, 128, DIM).transpose(1, 0, 2))


def build_host_constants(Wq, Wkv, Wproj, bproj, Wdwc, bdwc,
                         an_bias, na_bias, ah_bias, aw_bias, ha_bias, wa_bias):
    """Pure parameter re-layout / folding on host."""
    c = {}
    Wk, Wv = Wkv[:DIM], Wkv[DIM:]
    c["WqT"] = _wt_layout(Wq.T)
    c["WkT"] = _wt_layout(Wk.T)
    c["WvT"] = _wt_layout(Wv.T)
    c["WprojT"] = _wt_layout(Wproj.T)
    # pair-major Wproj^T rows: [64, 4, 256], pair p rows = dims 64p..64p+64
    c["WprojP"] = np.ascontiguousarray(
        Wproj.T.reshape(4, 64, DIM).transpose(1, 0, 2))

    Rh = _resize_matrix(7, H)          # [64, 7]
    Rw = _resize_matrix(7, W)
    # stage-1 bias: [n, (h, a64)]
    pb1 = np.einsum('yp,xq,hapq->hayx', Rh, Rw, an_bias)   # [h, a, y, x]
    b1 = pb1 + ah_bias[0][:, :, :, 0][:, :, :, None] \
        + aw_bias[0][:, :, 0, :][:, :, None, :]            # [h, a, y, x]
    bias1 = np.full((HEADS, AGP, N), PAD_BIAS, np.float32)
    bias1[:, :AGENT, :] = b1.reshape(HEADS, AGENT, N)
    bias1 = bias1.reshape(HAP, N).T                        # [n, 512]
    c["bias1"] = np.ascontiguousarray(
        bias1.reshape(NT // 2, 2, 128, HAP).transpose(0, 2, 1, 3))

    ab1 = np.einsum('yp,xq,hapq->hayx', Rh, Rw, na_bias)   # [h, a, y, x]
    # ha_bias [1,h,64,1,a] -> [h, a, y, 1]; wa_bias [1,h,1,64,a] -> [h, a, 1, x]
    b2 = ab1 + ha_bias[0, :, :, 0, :].transpose(0, 2, 1)[:, :, :, None] \
        + wa_bias[0, :, 0, :, :].transpose(0, 2, 1)[:, :, None, :]
    biasT2 = np.full((HEADS, AGP, N), PAD_BIAS, np.float32)
    biasT2[:, :AGENT, :] = b2.reshape(HEADS, AGENT, N)
    # [chunk, 128 (row within slice), pair-slice, 512]
    c["biasT2"] = np.ascontiguousarray(
        biasT2.reshape(HAP, N).reshape(NPAIR, 128, NCH, 512)
        .transpose(2, 1, 0, 3))

    binsH, binsW = _pool_bins(H, 7), _pool_bins(W, 7)
    s49 = np.zeros((1, AGENT), np.float32)
    for p in range(7):
        for q in range(7):
            lp = binsH[p][1] - binsH[p][0]
            lq = binsW[q][1] - binsW[q][0]
            s49[0, p * 7 + q] = 1.0 / (lp * lq)
    c["S49"] = s49

    w9 = Wdwc[:, 0, :, :].reshape(DIM, 9)
    diag = np.zeros((18, 128, 128), np.float32)
    for t in range(9):
        for ct_ in range(2):
            np.fill_diagonal(diag[ct_ * 9 + t], w9[ct_ * 128:(ct_ + 1) * 128, t])
    c["DIAG"] = np.ascontiguousarray(
        diag.reshape(2, 9, 128, 128).transpose(2, 0, 1, 3))  # [128, 2, 9, 128]
    c["W9"] = np.ascontiguousarray(
        w9.reshape(2, 128, 9).transpose(1, 0, 2))            # [128, 2, 9]

    bpe = (bproj + Wproj @ bdwc).astype(np.float32)
    c["bproj2"] = np.ascontiguousarray(bpe.reshape(2, 128).T)  # [128, 2]

    ind8 = np.zeros((128, NPAIR, 8), np.float32)
    for p in range(NPAIR):
        ind8[:64, p, 2 * p] = 1.0
        ind8[64:, p, 2 * p + 1] = 1.0
    c["ind8"] = ind8
    indp = np.zeros((8, NPAIR, 128), np.float32)
    for p in range(NPAIR):
        indp[2 * p, p, :64] = 1.0
        indp[2 * p + 1, p, 64:] = 1.0
    c["indp"] = indp
    c["ident"] = np.eye(128, dtype=np.float32)
    c["ones65"] = np.ones((65, 1), np.float32)
    c["zero1"] = np.zeros((1, 128), np.float32)
    c["ones1"] = np.ones((1, 128), np.float32)
    return c


CONST_SPECS = [
    ("WqT", BF16, [128, 2, DIM]),
    ("WkT", BF16, [128, 2, DIM]),
    ("WvT", BF16, [128, 2, DIM]),
    ("WprojT", BF16, [128, 2, DIM]),
    ("WprojP", BF16, [64, 4, DIM]),
    ("DIAG", BF16, [128, 2, 9, 128]),
    ("W9", F32, [128, 2, 9]),
    ("S49", F32, [1, AGENT]),
    ("bproj2", F32, [128, 2]),
    ("ind8", BF16, [128, NPAIR, 8]),
    ("indp", BF16, [8, NPAIR, 128]),
    ("ident", BF16, [128, 128]),
    ("ones65", F32, [65, 1]),
    ("zero1", BF16, [1, 128]),
    ("ones1", BF16, [1, 128]),
]
STREAM_SPECS = [
    ("bias1", BF16, [NT // 2, 128, 2, HAP]),
    ("biasT2", BF16, [NCH, 128, NPAIR, 512]),
]


def split_multiwaits(nc, max_waits=1):
    """Walrus codegen has one sync-wait slot per instruction; split extras
    into standalone EventSemaphore waits on the same engine."""
    n_split = 0
    for f in nc.m.functions:
        for bb in f.blocks:
            new_insts = []
            changed = False
            for inst in bb.instructions:
                si = inst.sync_info
                if (si is not None and si.on_wait is not None
                        and len(si.on_wait) > max_waits and inst.is_executable()):
                    waits = list(si.on_wait)
                    extra, keep = waits[:-max_waits], waits[-max_waits:]
                    for w in extra:
                        ev = mybir.InstEventSemaphore(
                            name=f"{inst.name}-ws{n_split}",
                            engine=inst.engine, ins=[], outs=[],
                            sync_info=mybir.SyncInfo(on_wait=[w], on_update=[]),
                        )
                        new_insts.append(ev)
                        n_split += 1
                    inst.sync_info = mybir.SyncInfo(
                        on_wait=keep, on_update=list(si.on_update))
                    changed = True
                new_insts.append(inst)
            if changed:
                bb.instructions = new_insts
    return n_split


def build_nc():
    nc = bass.Bass()
    x_in = nc.dram_tensor("x", [SPC, DIM, N], BF16, kind="ExternalInput")
    out_d = nc.dram_tensor("out", [SPC, DIM, N], F16, kind="ExternalOutput")
    cst = {name: nc.dram_tensor(name, shape, dt, kind="ExternalInput")
           for name, dt, shape in CONST_SPECS + STREAM_SPECS}

    with tile.TileContext(nc) as tc, ExitStack() as ctx:
        kernel_body(ctx, tc, nc, x_in, out_d, cst)
    split_multiwaits(nc)
    return nc


def kernel_body(ctx, tc, nc, x_in, out_d, cst):
    const = ctx.enter_context(tc.tile_pool(name="const", bufs=1))
    big = ctx.enter_context(tc.tile_pool(name="big", bufs=1))
    work = ctx.enter_context(tc.tile_pool(name="work", bufs=3))
    small = ctx.enter_context(tc.tile_pool(name="small", bufs=2))
    outp = ctx.enter_context(tc.tile_pool(name="outp", bufs=3))
    # PSUM budget (8 banks): ps_main x3, ps_out x2, ps_s2/ps_r2/ps_av x1
    dram = ctx.enter_context(tc.tile_pool(name="dram", bufs=2, space="DRAM"))
    psA = ctx.enter_context(tc.tile_pool(name="psA", bufs=3, space="PSUM"))
    psB = ctx.enter_context(tc.tile_pool(name="psB", bufs=2, space="PSUM"))
    psAcc = ctx.enter_context(tc.tile_pool(name="psAcc", bufs=1, space="PSUM"))

    ct = {}
    for name, dt, shape in CONST_SPECS:
        t = const.tile(shape, dt, name=f"c_{name}")
        nc.sync.dma_start(out=t, in_=cst[name].ap())
        ct[name] = t
    s49rep = const.tile([128, AGENT], F32, name="s49rep")
    nc.sync.dma_start(
        out=s49rep,
        in_=bass.AP(tensor=cst["S49"], offset=0, ap=[[0, 128], [1, AGENT]]))
    ct["s49rep"] = s49rep
    ct["bias1_d"] = cst["bias1"]
    ct["biasT2_d"] = cst["biasT2"]

    # persistent zero-padded agent_v blockdiag lhsT [64, 4, 128]
    avbd = const.tile([64, NPAIR, 128], BF16, name="avbd")
    nc.vector.memset(avbd, 0.0)
    ct["avbd"] = avbd
    # vtok5: [128, NT, 4, 65] token-major V with per-pair ones column
    vtok5 = big.tile([128, NT, NPAIR, 65], BF16, name="vtok5", tag="vtok5")
    nc.vector.memset(
        bass.AP(tensor=vtok5.tensor, offset=vtok5.offset + 64,
                ap=[vtok5.ap[0], [NPAIR * 65, NT], [65, NPAIR], [1, 1]]), 1.0)
    ct["vtok5"] = vtok5

    def load_x(s):
        row = []
        for hf in range(2):
            t = big.tile([128, N], BF16, name=f"x{s}{hf}", tag=f"xh{hf}")
            nc.sync.dma_start(
                out=t, in_=x_in.ap()[s, hf * 128:(hf + 1) * 128, :])
            row.append(t)
        return row

    xh = load_x(0)
    for s in range(SPC):
        xh = sample(nc, ct, s, xh, load_x, x_in, out_d, big, work, small,
                    outp, psA, psB, psAcc, dram)


def sample(nc, ct, s, xh, load_x, x_in, out_d, big, work, small, outp,
           psA, psB, psAcc, dram):
    F = 512
    vtok5 = ct["vtok5"]

    # ---- QKV projections (dim-major) ------------------------------------
    QT, KT, VT = [], [], []
    for hf in range(2):
        QT.append(big.tile([128, N], BF16, name=f"qt{hf}", tag=f"qt{hf}"))
        KT.append(big.tile([128, N], BF16, name=f"kt{hf}", tag=f"kt{hf}"))
        VT.append(big.tile([128, N], BF16, name=f"v{hf}", tag=f"v{hf}"))

    for wname, dest in (("WvT", "v"), ("WkT", "k"), ("WqT", "q")):
        wt = ct[wname]
        for mt in range(2):
            for chn in range(NCH):
                ps = psA.tile([128, F], F32, name="ps_proj", tag="ps_main")
                for kt_ in range(2):
                    nc.tensor.matmul(
                        ps,
                        lhsT=wt[:, kt_, mt * 128:(mt + 1) * 128],
                        rhs=xh[kt_][:, chn * F:(chn + 1) * F],
                        start=(kt_ == 0), stop=(kt_ == 1))
                if dest == "q":
                    nc.vector.tensor_copy(
                        out=QT[mt][:, chn * F:(chn + 1) * F], in_=ps)
                elif dest == "k":
                    nc.vector.tensor_copy(
                        out=KT[mt][:, chn * F:(chn + 1) * F], in_=ps)
                else:
                    nc.scalar.copy(out=VT[mt][:, chn * F:(chn + 1) * F], in_=ps)

    # ---- token-major V (direct projection, no transposes) ---------------
    for t in range(NT):
        ps = psB.tile([128, DIM], F32, name="ps_vt", tag="ps_out", bufs=2)
        for kt_ in range(2):
            nc.tensor.matmul(
                ps, lhsT=xh[kt_][:, t * 128:(t + 1) * 128],
                rhs=ct["WvT"][:, kt_, :],
                start=(kt_ == 0), stop=(kt_ == 1))
        nc.vector.tensor_copy(
            out=bass.AP(tensor=vtok5.tensor,
                        offset=vtok5.offset + t * NPAIR * 65,
                        ap=[vtok5.ap[0], [65, NPAIR], [1, 64]]),
            in_=ps.rearrange("p (a b) -> p a b", a=NPAIR))

    # ---- agent tokens: pool X -> project -> blockdiag (gpsimd pooling) --
    binsH, binsW = _pool_bins(H, 7), _pool_bins(W, 7)
    XpH = []
    for hf in range(2):
        x3 = xh[hf].rearrange("p (y x) -> p y x", y=H)
        qx = small.tile([128, H, 7], F32, name="qx", tag="qx")
        for q, (s0, e0) in enumerate(binsW):
            nc.vector.tensor_reduce(
                out=qx[:, :, q:q + 1], in_=x3[:, :, s0:e0],
                axis=AX.X, op=mybir.AluOpType.add)
        xp = small.tile([128, 7, 7], F32, name="xp", tag="xp")
        qxf = qx.rearrange("p y q -> p (y q)")
        for p, (s0, e0) in enumerate(binsH):
            seg = bass.AP(tensor=qxf.tensor, offset=qxf.offset + s0 * 7,
                          ap=[qxf.ap[0], [1, 7], [7, e0 - s0]])
            nc.vector.tensor_reduce(
                out=xp[:, p, :], in_=seg, axis=AX.X, op=mybir.AluOpType.add)
        xpb = small.tile([128, AGENT], BF16, name="xpb", tag="xpb")
        nc.vector.tensor_mul(
            out=xpb, in0=xp.rearrange("p a b -> p (a b)"), in1=ct["s49rep"])
        XpH.append(xpb)

    agentT = []
    for mt in range(2):
        ps = psB.tile([128, AGENT], F32, name="ps_ag", tag="ps_out", bufs=2)
        for kt_ in range(2):
            nc.tensor.matmul(
                ps,
                lhsT=ct["WqT"][:, kt_, mt * 128:(mt + 1) * 128],
                rhs=XpH[kt_], start=(kt_ == 0), stop=(kt_ == 1))
        at = small.tile([128, AGENT], BF16, name=f"at{mt}", tag=f"at{mt}")
        nc.scalar.activation(out=at, in_=ps, func=AF.Copy, scale=SCALE)
        agentT.append(at)

    bd = []
    for hf in range(2):
        b = small.tile([128, 4 * AGP], BF16, name=f"bd{hf}", tag=f"bd{hf}")
        nc.vector.memset(b, 0.0)
        for hl in range(4):
            nc.vector.tensor_copy(
                out=b[hl * 32:(hl + 1) * 32, hl * AGP:hl * AGP + AGENT],
                in_=agentT[hf][hl * 32:(hl + 1) * 32, :])
        bd.append(b)

    # prefetch next sample's input; same tags, so the DMA waits for this
    # sample's last xh reader and overlaps the rest of this sample
    xh_next = load_x(s + 1) if s + 1 < SPC else None

    # ---- dwc: diagonal matmuls, x-wrap columns fixed on DVE -------------
    DWall = big.tile([128, 2, N], BF16, name="dwall", tag="dwall")
    TAPS = [(0, 0)] + [(dy, dx) for dy in (-1, 0, 1) for dx in (-1, 0, 1)
                       if (dy, dx) != (0, 0)]
    for cti in range(2):
        v = VT[cti]
        for chn in range(NCH):
            ps = psA.tile([128, F], F32, name="ps_dw", tag="ps_main")
            lo = chn * F
            for k, (dy, dx) in enumerate(TAPS):
                t9 = (dy + 1) * 3 + (dx + 1)
                d = dy * W + dx
                a = max(0, -(lo + d))
                b_ = max(0, (lo + F + d) - N)
                nc.tensor.matmul(
                    ps[:, a:F - b_], lhsT=ct["DIAG"][:, cti, t9, :],
                    rhs=v[:, lo + d + a:lo + F + d - b_],
                    start=(k == 0), stop=(k == 8), skip_group_check=True)
            nc.scalar.copy(out=DWall[:, cti, lo:lo + F], in_=ps)
        # x-boundary columns: recompute exactly with strided DVE ops
        dwp = DWall[:, cti, :]
        for xb, dxs in ((0, (0, 1)), (W - 1, (-1, 0))):
            first = True
            for dy in (0, -1, 1):     # dy=0 first: full row range overwrite
                for dx in dxs:
                    t9 = (dy + 1) * 3 + (dx + 1)
                    rs, re = max(0, -dy), H - max(0, dy)
                    nr = re - rs
                    o_ap = bass.AP(tensor=dwp.tensor,
                                   offset=dwp.offset + rs * W + xb,
                                   ap=[dwp.ap[0], [W, nr]])
                    v_ap = bass.AP(tensor=v.tensor,
                                   offset=v.offset + (rs + dy) * W + xb + dx,
                                   ap=[v.ap[0], [W, nr]])
                    wcol = ct["W9"][:, cti, t9:t9 + 1]
                    if first:
                        nc.vector.tensor_scalar_mul(
                            out=o_ap, in0=v_ap, scalar1=wcol)
                        first = False
                    else:
                        nc.vector.scalar_tensor_tensor(
                            out=o_ap, in0=v_ap, scalar=wcol, in1=o_ap,
                            op0=mybir.AluOpType.mult, op1=mybir.AluOpType.add)

    # ---- stage 1: agent -> kv attention (token-major) -------------------
    ps_av = psAcc.tile([65, 512], F32, name="ps_av", tag="ps_av")
    # clear has_written bits for the whole bank
    nc.tensor.matmul(ps_av[0:65, 0:128], lhsT=ct["zero1"][:, 0:65],
                     rhs=ct["ones1"], start=True, stop=False,
                     skip_group_check=True)
    # av8 matmuls are skewed one tile behind the score/exp pipeline so the
    # PE never waits on the exp of the tile it just scored.
    e1s = {}
    btps = {}
    for t in range(NT + 1):
        if t < NT:
            if t % 2 == 0:
                btp = work.tile([128, 2, HAP], BF16, name="bt", tag="bt")
                nc.sync.dma_start(out=btp, in_=ct["bias1_d"].ap()[t // 2])
                btps[t // 2] = btp
            btp = btps[t // 2]
            ps = psA.tile([128, HAP], F32, name="ps_s1", tag="ps_main")
            nc.tensor.matmul(ps, lhsT=ct["ident"], rhs=btp[:, t % 2, :],
                             start=True, stop=False, skip_group_check=True)
            for hf in range(2):
                nc.tensor.matmul(
                    ps[:, hf * 256:(hf + 1) * 256],
                    lhsT=KT[hf][:, t * 128:(t + 1) * 128],
                    rhs=bd[hf], start=False, stop=(hf == 1),
                    skip_group_check=True)
            e1 = work.tile([128, HAP], BF16, name="e1", tag="e1")
            nc.scalar.activation(out=e1, in_=ps, func=AF.Exp)
            e1s[t] = e1
        if t >= 1:
            e1 = e1s.pop(t - 1)
            for p in range(NPAIR):
                nc.tensor.matmul(
                    ps_av[0:65, p * 128:(p + 1) * 128],
                    lhsT=vtok5[:, t - 1, p, :],
                    rhs=e1[:, p * 128:(p + 1) * 128],
                    start=False, stop=(t - 1 == NT - 1),
                    skip_group_check=True)

    # ---- stage 2 (agent-major) + fused output, 2-chunk software pipeline
    a2n = big.tile([128, NPAIR, N], BF16, name="a2n", tag="a2n")
    WAV = small.tile([128, NPAIR, DIM], BF16, name="wav", tag="wav")

    def s1_post():
        # colsum row -> SBUF, transpose per pair into [128,4], reciprocal
        cs1 = small.tile([65, HAP], F32, name="cs1", tag="cs1")
        nc.scalar.copy(out=cs1[64:65, :], in_=ps_av[64:65, :])
        ps4 = psB.tile([128, NPAIR], F32, name="ps_cs4", tag="ps_s2",
                       bufs=1)
        for p in range(NPAIR):
            nc.tensor.transpose(
                ps4[:, p:p + 1], in_=cs1[64:65, p * 128:(p + 1) * 128],
                identity=ct["ones65"][64:65, :])
        rcp4 = small.tile([128, NPAIR], F32, name="rcp4", tag="rcp4")
        nc.vector.reciprocal(out=rcp4, in_=ps4)
        avbd = ct["avbd"]
        for p in range(NPAIR):
            nc.scalar.copy(
                out=avbd[0:32, p, 0:AGENT],
                in_=ps_av[0:32, p * 128:p * 128 + AGENT])
            nc.scalar.copy(
                out=avbd[32:64, p, 64:64 + AGENT],
                in_=ps_av[32:64, p * 128 + 64:p * 128 + 64 + AGENT])
        for p in range(NPAIR):
            psW = psB.tile([128, DIM], F32, name="ps_wav", tag="ps_out", bufs=2)
            nc.tensor.matmul(psW, lhsT=avbd[:, p, :],
                             rhs=ct["WprojP"][:, p, :], start=True, stop=True)
            nc.scalar.activation(out=WAV[:, p, :], in_=psW, func=AF.Copy,
                                 scale=rcp4[:, p:p + 1])

    def s2_scores(c):
        bt2 = work.tile([128, NPAIR, F], BF16, name="btT", tag="bt")
        nc.sync.dma_start(out=bt2, in_=ct["biasT2_d"].ap()[c])
        for p in range(NPAIR):
            ps = psA.tile([128, F], F32, name="ps_s2", tag="ps_main")
            nc.tensor.matmul(ps, lhsT=ct["ident"], rhs=bt2[:, p, :],
                             start=True, stop=False, skip_group_check=True)
            nc.tensor.matmul(
                ps, lhsT=bd[p // 2][:, (p % 2) * 128:(p % 2) * 128 + 128],
                rhs=QT[p // 2][:, c * F:(c + 1) * F],
                start=False, stop=True, skip_group_check=True)
            nc.scalar.activation(out=a2n[:, p, c * F:(c + 1) * F], in_=ps,
                                 func=AF.Exp)
        ps_s2 = psB.tile([8, F], F32, name="ps_den", tag="ps_s2", bufs=1)
        for p in range(NPAIR):
            nc.tensor.matmul(
                ps_s2, lhsT=ct["ind8"][:, p, :],
                rhs=a2n[:, p, c * F:(c + 1) * F],
                start=(p == 0), stop=(p == NPAIR - 1))
        # r2 = 1/s2 via exp(-ln(s2)) on the scalar engine (s2 in [46, 53])
        r2f = small.tile([8, F], F32, name="r2f", tag="r2f")
        nc.scalar.activation(out=r2f, in_=ps_s2, func=AF.Ln)
        r2c = small.tile([8, F], BF16, name="r2c", tag="r2c")
        nc.scalar.activation(out=r2c, in_=r2f, func=AF.Exp, scale=-1.0)
        return r2c

    def s2_norm(c, r2c):
        for p in range(NPAIR):
            ps_r2 = psB.tile([128, F], F32, name="ps_r2", tag="ps_r2",
                             bufs=1)
            nc.tensor.matmul(ps_r2, lhsT=ct["indp"][:, p, :], rhs=r2c,
                             start=True, stop=True)
            sl = a2n[:, p, c * F:(c + 1) * F]
            nc.vector.tensor_mul(out=sl, in0=sl, in1=ps_r2)

    def s2_out(c):
        for ot in range(2):
            ps_o = psB.tile([128, F], F32, name="ps_o", tag="ps_out", bufs=2)
            for p in range(NPAIR):
                nc.tensor.matmul(
                    ps_o, lhsT=WAV[:, p, ot * 128:(ot + 1) * 128],
                    rhs=a2n[:, p, c * F:(c + 1) * F],
                    start=(p == 0), stop=False, skip_group_check=True)
            for kt_ in range(2):
                nc.tensor.matmul(
                    ps_o,
                    lhsT=ct["WprojT"][:, kt_, ot * 128:(ot + 1) * 128],
                    rhs=DWall[:, kt_, c * F:(c + 1) * F],
                    start=False, stop=(kt_ == 1), skip_group_check=True)
            o = outp.tile([128, F], F16, name="o_st", tag="o_st")
            nc.vector.tensor_scalar_add(
                out=o, in0=ps_o, scalar1=ct["bproj2"][:, ot:ot + 1])
            nc.gpsimd.dma_start(
                out=out_d.ap()[s, ot * 128:(ot + 1) * 128,
                               c * F:(c + 1) * F],
                in_=o)

    r2cs = {}
    for c in range(NCH + 2):
        if c < NCH:
            r2cs[c] = s2_scores(c)
        if c == 0:
            s1_post()
        if 1 <= c <= NCH:
            s2_norm(c - 1, r2cs.pop(c - 1))
        if c >= 2:
            s2_out(c - 2)
    return xh_next


def kernel(**inputs):
    x = np.asarray(inputs["x"], np.float32)
    host = build_host_constants(
        np.asarray(inputs["Wq"], np.float32),
        np.asarray(inputs["Wkv"], np.float32),
        np.asarray(inputs["Wproj"], np.float32),
        np.asarray(inputs["bproj"], np.float32),
        np.asarray(inputs["Wdwc"], np.float32),
        np.asarray(inputs["bdwc"], np.float32),
        np.asarray(inputs["an_bias"], np.float32),
        np.asarray(inputs["na_bias"], np.float32),
        np.asarray(inputs["ah_bias"], np.float32),
        np.asarray(inputs["aw_bias"], np.float32),
        np.asarray(inputs["ha_bias"], np.float32),
        np.asarray(inputs["wa_bias"], np.float32),
    )
    nc = build_nc()

    specs = CONST_SPECS + STREAM_SPECS
    np_dt = {mybir.dt.float32: np.float32, mybir.dt.float16: np.float16,
             mybir.dt.bfloat16: ml_dtypes.bfloat16}

    const_map = {name: np.asarray(host[name], np_dt[dt])
                 for name, dt, _ in specs}
    xs = x.reshape(B, DIM, N)
    in_maps = []
    for c in range(N_CORES):
        m = dict(const_map)
        m["x"] = np.asarray(xs[c * SPC:(c + 1) * SPC], ml_dtypes.bfloat16)
        in_maps.append(m)

    res = run_bass_kernel_spmd(nc, in_maps, core_ids=list(range(N_CORES)))
    out = np.concatenate([res.results[c]["out"] for c in range(N_CORES)],
                         axis=0)
    return out.astype(np.float32).reshape(B, DIM, H, W)


# revision 22
# speedup vs baseline: 1.1409x; 1.0195x over previous
"""AgentAttention Trainium2 kernel (v2).

Full inputs -> shard batch over 8 NeuronCores (2 samples each) -> Bass/Tile
kernel per core -> gather full output.

v2 redesign vs baseline:
- All PE operands bf16 (moving-operand fast path), fp16 output DMA.
- Position biases fully precomputed on host ([N,512] stage-1 and [512,N]
  stage-2 tables) and injected per tile via a single identity matmul that
  initializes the score PSUM -- replaces the 2-matmul Phi-basis rebuild.
- V is projected twice: dim-major (for dwc) and token-major directly
  (lhsT=X trick), eliminating all per-tile PE transposes of V.
- Stage-1 agent_v computed in [dim, agent] orientation with a ones column
  appended to the vtok lhsT, so the softmax denominators (column sums)
  drop out of the same matmuls; normalization folds into the WAV
  (Wproj @ agent_v) evacuation as a per-partition ACT scale.
- Stage-2 runs agent-major ([ha, n] scores via lhsT=agent-blockdiag,
  rhs=Q), eliminating the 4-per-tile PE transposes of the prob matrix.
  Softmax denominators come from indicator matmuls ([8,512] PSUM), a fast
  approximate reciprocal, and a k=8 broadcast matmul re-expands them.
- Output matmul consumes [ha, n] probs + dwc result; bproj (+ Wproj@bdwc)
  is added during the output evacuation as a per-partition DVE scalar.
- 3x3 depthwise conv stays as PE diagonal matmuls with flat shifts; the
  x-wrap columns are recomputed exactly by DVE strided ops.
"""
import sys
import numpy as np
from contextlib import ExitStack

sys.path.insert(0, "/opt/trn_rl_repo")

import ml_dtypes
import concourse.bass as bass
import concourse.tile as tile
from concourse import mybir
from concourse.bass_utils import run_bass_kernel_spmd

B, DIM, HEADS, AGENT = 16, 256, 8, 49
H = W = 64
N = H * W
HD = DIM // HEADS             # 32
SCALE = HD ** -0.5
N_CORES = 8
SPC = B // N_CORES            # 2 samples per core
NT = N // 128                 # 32 token tiles
NCH = N // 512                # 8 token chunks
AGP = 64                      # padded agent dim per head
HAP = HEADS * AGP             # 512
NPAIR = 4
PAD_BIAS = -30.0

F32 = mybir.dt.float32
F16 = mybir.dt.float16
BF16 = mybir.dt.bfloat16
AX = mybir.AxisListType
AF = mybir.ActivationFunctionType


def _pool_bins(size, out):
    return [((i * size) // out, -((-(i + 1) * size) // out)) for i in range(out)]


def _resize_matrix(n_in, n_out):
    """Row-normalized half-pixel triangle-kernel resize matrix: matches
    jax.image.resize(method='bilinear') upsampling."""
    R = np.zeros((n_out, n_in), np.float64)
    scale = n_in / n_out
    for o in range(n_out):
        c = (o + 0.5) * scale - 0.5
        for i in range(n_in):
            R[o, i] = max(0.0, 1.0 - abs(c - i))
        s = R[o].sum()
        if s > 0:
            R[o] /= s
    return R.astype(np.float32)


def _wt_layout(WT):
    """[256, 256] (cin, cout) -> [128, 2, 256] tile layout, [p, k, m]."""
    return np.ascontiguousarray(WT.reshape(2, 128, DIM).transpose(1, 0, 2))


def build_host_constants(Wq, Wkv, Wproj, bproj, Wdwc, bdwc,
                         an_bias, na_bias, ah_bias, aw_bias, ha_bias, wa_bias):
    """Pure parameter re-layout / folding on host."""
    c = {}
    Wk, Wv = Wkv[:DIM], Wkv[DIM:]
    c["WqT"] = _wt_layout(Wq.T)
    c["WkT"] = _wt_layout(Wk.T)
    c["WvT"] = _wt_layout(Wv.T)
    c["WprojT"] = _wt_layout(Wproj.T)
    # pair-major Wproj^T rows: [64, 4, 256], pair p rows = dims 64p..64p+64
    c["WprojP"] = np.ascontiguousarray(
        Wproj.T.reshape(4, 64, DIM).transpose(1, 0, 2))

    Rh = _resize_matrix(7, H)          # [64, 7]
    Rw = _resize_matrix(7, W)
    # stage-1 bias: [n, (h, a64)]
    pb1 = np.einsum('yp,xq,hapq->hayx', Rh, Rw, an_bias)   # [h, a, y, x]
    b1 = pb1 + ah_bias[0][:, :, :, 0][:, :, :, None] \
        + aw_bias[0][:, :, 0, :][:, :, None, :]            # [h, a, y, x]
    bias1 = b1.reshape(HEADS * AGENT, N).T                 # [n, 392]
    c["bias1"] = np.ascontiguousarray(
        bias1.reshape(NT // 2, 2, 128, HEADS * AGENT).transpose(0, 2, 1, 3))

    ab1 = np.einsum('yp,xq,hapq->hayx', Rh, Rw, na_bias)   # [h, a, y, x]
    # ha_bias [1,h,64,1,a] -> [h, a, y, 1]; wa_bias [1,h,1,64,a] -> [h, a, 1, x]
    b2 = ab1 + ha_bias[0, :, :, 0, :].transpose(0, 2, 1)[:, :, :, None] \
        + wa_bias[0, :, 0, :, :].transpose(0, 2, 1)[:, :, None, :]
    biasT2 = np.full((HEADS, AGP, N), PAD_BIAS, np.float32)
    biasT2[:, :AGENT, :] = b2.reshape(HEADS, AGENT, N)
    # [chunk, 128 (row within slice), pair-slice, 512]
    c["biasT2"] = np.ascontiguousarray(
        biasT2.reshape(HAP, N).reshape(NPAIR, 128, NCH, 512)
        .transpose(2, 1, 0, 3))

    binsH, binsW = _pool_bins(H, 7), _pool_bins(W, 7)
    s49 = np.zeros((1, AGENT), np.float32)
    for p in range(7):
        for q in range(7):
            lp = binsH[p][1] - binsH[p][0]
            lq = binsW[q][1] - binsW[q][0]
            s49[0, p * 7 + q] = 1.0 / (lp * lq)
    c["S49"] = s49

    w9 = Wdwc[:, 0, :, :].reshape(DIM, 9)
    diag = np.zeros((18, 128, 128), np.float32)
    for t in range(9):
        for ct_ in range(2):
            np.fill_diagonal(diag[ct_ * 9 + t], w9[ct_ * 128:(ct_ + 1) * 128, t])
    c["DIAG"] = np.ascontiguousarray(
        diag.reshape(2, 9, 128, 128).transpose(2, 0, 1, 3))  # [128, 2, 9, 128]
    c["W9"] = np.ascontiguousarray(
        w9.reshape(2, 128, 9).transpose(1, 0, 2))            # [128, 2, 9]

    bpe = (bproj + Wproj @ bdwc).astype(np.float32)
    c["bproj2"] = np.ascontiguousarray(bpe.reshape(2, 128).T)  # [128, 2]

    ind8 = np.zeros((128, NPAIR, 8), np.float32)
    for p in range(NPAIR):
        ind8[:64, p, 2 * p] = 1.0
        ind8[64:, p, 2 * p + 1] = 1.0
    c["ind8"] = ind8
    indp = np.zeros((8, NPAIR, 128), np.float32)
    for p in range(NPAIR):
        indp[2 * p, p, :64] = 1.0
        indp[2 * p + 1, p, 64:] = 1.0
    c["indp"] = indp
    c["ident"] = np.eye(128, dtype=np.float32)
    c["ones65"] = np.ones((65, 1), np.float32)
    c["zero1"] = np.zeros((1, 128), np.float32)
    c["ones1"] = np.ones((1, 128), np.float32)
    return c


CONST_SPECS = [
    ("WvT", BF16, [128, 2, DIM]),
    ("WkT", BF16, [128, 2, DIM]),
    ("WqT", BF16, [128, 2, DIM]),
    ("WprojT", BF16, [128, 2, DIM]),
    ("WprojP", BF16, [64, 4, DIM]),
    ("DIAG", BF16, [128, 2, 9, 128]),
    ("W9", F32, [128, 2, 9]),
    ("S49", F32, [1, AGENT]),
    ("bproj2", F32, [128, 2]),
    ("ind8", BF16, [128, NPAIR, 8]),
    ("indp", BF16, [8, NPAIR, 128]),
    ("ident", BF16, [128, 128]),
    ("ones65", F32, [65, 1]),
    ("zero1", BF16, [1, 128]),
    ("ones1", BF16, [1, 128]),
]
STREAM_SPECS = [
    ("bias1", BF16, [NT // 2, 128, 2, HEADS * AGENT]),
    ("biasT2", BF16, [NCH, 128, NPAIR, 512]),
]


def split_multiwaits(nc, max_waits=1):
    """Walrus codegen has one sync-wait slot per instruction; split extras
    into standalone EventSemaphore waits on the same engine."""
    n_split = 0
    for f in nc.m.functions:
        for bb in f.blocks:
            new_insts = []
            changed = False
            for inst in bb.instructions:
                si = inst.sync_info
                if (si is not None and si.on_wait is not None
                        and len(si.on_wait) > max_waits and inst.is_executable()):
                    waits = list(si.on_wait)
                    extra, keep = waits[:-max_waits], waits[-max_waits:]
                    for w in extra:
                        ev = mybir.InstEventSemaphore(
                            name=f"{inst.name}-ws{n_split}",
                            engine=inst.engine, ins=[], outs=[],
                            sync_info=mybir.SyncInfo(on_wait=[w], on_update=[]),
                        )
                        new_insts.append(ev)
                        n_split += 1
                    inst.sync_info = mybir.SyncInfo(
                        on_wait=keep, on_update=list(si.on_update))
                    changed = True
                new_insts.append(inst)
            if changed:
                bb.instructions = new_insts
    return n_split


def build_nc():
    nc = bass.Bass()
    x_in = nc.dram_tensor("x", [SPC, DIM, N], BF16, kind="ExternalInput")
    out_d = nc.dram_tensor("out", [SPC, DIM, N], F16, kind="ExternalOutput")
    cst = {name: nc.dram_tensor(name, shape, dt, kind="ExternalInput")
           for name, dt, shape in CONST_SPECS + STREAM_SPECS}

    with tile.TileContext(nc) as tc, ExitStack() as ctx:
        kernel_body(ctx, tc, nc, x_in, out_d, cst)
    split_multiwaits(nc)
    return nc


def kernel_body(ctx, tc, nc, x_in, out_d, cst):
    const = ctx.enter_context(tc.tile_pool(name="const", bufs=1))
    big = ctx.enter_context(tc.tile_pool(name="big", bufs=1))
    work = ctx.enter_context(tc.tile_pool(name="work", bufs=3))
    small = ctx.enter_context(tc.tile_pool(name="small", bufs=2))
    outp = ctx.enter_context(tc.tile_pool(name="outp", bufs=3))
    # PSUM budget (8 banks): ps_main x3, ps_out x2, ps_s2/ps_r2/ps_av x1
    dram = ctx.enter_context(tc.tile_pool(name="dram", bufs=2, space="DRAM"))
    psA = ctx.enter_context(tc.tile_pool(name="psA", bufs=3, space="PSUM"))
    psB = ctx.enter_context(tc.tile_pool(name="psB", bufs=2, space="PSUM"))
    psAcc = ctx.enter_context(tc.tile_pool(name="psAcc", bufs=1, space="PSUM"))

    def load_x(s):
        row = []
        for hf in range(2):
            t = big.tile([128, N], BF16, name=f"x{s}{hf}", tag=f"xh{hf}")
            nc.sync.dma_start(
                out=t, in_=x_in.ap()[s, hf * 128:(hf + 1) * 128, :])
            row.append(t)
        return row

    xh0 = load_x(0)
    ct = {}
    for name, dt, shape in CONST_SPECS:
        t = const.tile(shape, dt, name=f"c_{name}")
        nc.sync.dma_start(out=t, in_=cst[name].ap())
        ct[name] = t
    s49rep = const.tile([128, AGENT], F32, name="s49rep")
    nc.sync.dma_start(
        out=s49rep,
        in_=bass.AP(tensor=cst["S49"], offset=0, ap=[[0, 128], [1, AGENT]]))
    ct["s49rep"] = s49rep
    ct["bias1_d"] = cst["bias1"]
    ct["biasT2_d"] = cst["biasT2"]

    # persistent zero-padded agent_v blockdiag lhsT [64, 4, 128]
    avbd = const.tile([64, NPAIR, 128], BF16, name="avbd")
    nc.vector.memset(avbd, 0.0)
    ct["avbd"] = avbd
    # vtok5: [128, NT, 4, 65] token-major V with per-pair ones column
    vtok5 = big.tile([128, NT, NPAIR, 65], BF16, name="vtok5", tag="vtok5")
    nc.vector.memset(
        bass.AP(tensor=vtok5.tensor, offset=vtok5.offset + 64,
                ap=[vtok5.ap[0], [NPAIR * 65, NT], [65, NPAIR], [1, 1]]), 1.0)
    ct["vtok5"] = vtok5

    xh = xh0
    for s in range(SPC):
        xh = sample(nc, ct, s, xh, load_x, x_in, out_d, big, work, small,
                    outp, psA, psB, psAcc, dram)


def sample(nc, ct, s, xh, load_x, x_in, out_d, big, work, small, outp,
           psA, psB, psAcc, dram):
    F = 512
    vtok5 = ct["vtok5"]

    # ---- QKV projections (dim-major) ------------------------------------
    QT, KT, VT = [], [], []
    for hf in range(2):
        QT.append(big.tile([128, N], BF16, name=f"qt{hf}", tag=f"qt{hf}"))
        KT.append(big.tile([128, N], BF16, name=f"kt{hf}", tag=f"kt{hf}"))
        VT.append(big.tile([128, N], BF16, name=f"v{hf}", tag=f"v{hf}"))

    for wname, dest in (("WvT", "v"), ("WkT", "k"), ("WqT", "q")):
        wt = ct[wname]
        for mt in range(2):
            for chn in range(NCH):
                ps = psA.tile([128, F], F32, name="ps_proj", tag="ps_main")
                for kt_ in range(2):
                    nc.tensor.matmul(
                        ps,
                        lhsT=wt[:, kt_, mt * 128:(mt + 1) * 128],
                        rhs=xh[kt_][:, chn * F:(chn + 1) * F],
                        start=(kt_ == 0), stop=(kt_ == 1))
                if dest == "q":
                    nc.vector.tensor_copy(
                        out=QT[mt][:, chn * F:(chn + 1) * F], in_=ps)
                elif dest == "k":
                    nc.vector.tensor_copy(
                        out=KT[mt][:, chn * F:(chn + 1) * F], in_=ps)
                else:
                    nc.scalar.copy(out=VT[mt][:, chn * F:(chn + 1) * F], in_=ps)

    # ---- token-major V (direct projection, no transposes) ---------------
    for t in range(NT):
        ps = psB.tile([128, DIM], F32, name="ps_vt", tag="ps_out", bufs=2)
        for kt_ in range(2):
            nc.tensor.matmul(
                ps, lhsT=xh[kt_][:, t * 128:(t + 1) * 128],
                rhs=ct["WvT"][:, kt_, :],
                start=(kt_ == 0), stop=(kt_ == 1))
        nc.vector.tensor_copy(
            out=bass.AP(tensor=vtok5.tensor,
                        offset=vtok5.offset + t * NPAIR * 65,
                        ap=[vtok5.ap[0], [65, NPAIR], [1, 64]]),
            in_=ps.rearrange("p (a b) -> p a b", a=NPAIR))

    # ---- agent tokens: pool X -> project -> blockdiag (gpsimd pooling) --
    binsH, binsW = _pool_bins(H, 7), _pool_bins(W, 7)
    XpH = []
    for hf in range(2):
        x3 = xh[hf].rearrange("p (y x) -> p y x", y=H)
        qx = small.tile([128, H, 7], F32, name="qx", tag="qx")
        for q, (s0, e0) in enumerate(binsW):
            nc.vector.tensor_reduce(
                out=qx[:, :, q:q + 1], in_=x3[:, :, s0:e0],
                axis=AX.X, op=mybir.AluOpType.add)
        xp = small.tile([128, 7, 7], F32, name="xp", tag="xp")
        qxf = qx.rearrange("p y q -> p (y q)")
        for p, (s0, e0) in enumerate(binsH):
            seg = bass.AP(tensor=qxf.tensor, offset=qxf.offset + s0 * 7,
                          ap=[qxf.ap[0], [1, 7], [7, e0 - s0]])
            nc.vector.tensor_reduce(
                out=xp[:, p, :], in_=seg, axis=AX.X, op=mybir.AluOpType.add)
        xpb = small.tile([128, AGENT], BF16, name="xpb", tag="xpb")
        nc.vector.tensor_mul(
            out=xpb, in0=xp.rearrange("p a b -> p (a b)"), in1=ct["s49rep"])
        XpH.append(xpb)

    agentT = []
    for mt in range(2):
        ps = psB.tile([128, AGENT], F32, name="ps_ag", tag="ps_out", bufs=2)
        for kt_ in range(2):
            nc.tensor.matmul(
                ps,
                lhsT=ct["WqT"][:, kt_, mt * 128:(mt + 1) * 128],
                rhs=XpH[kt_], start=(kt_ == 0), stop=(kt_ == 1))
        at = small.tile([128, AGENT], BF16, name=f"at{mt}", tag=f"at{mt}")
        nc.scalar.activation(out=at, in_=ps, func=AF.Copy, scale=SCALE)
        agentT.append(at)

    bd = []
    bd49 = []
    for hf in range(2):
        b = small.tile([128, 4 * AGP], BF16, name=f"bd{hf}", tag=f"bd{hf}")
        nc.vector.memset(b, 0.0)
        b49 = small.tile([128, 4 * AGENT], BF16, name=f"bd49{hf}",
                         tag=f"bd49{hf}")
        nc.vector.memset(b49, 0.0)
        for hl in range(4):
            nc.vector.tensor_copy(
                out=b[hl * 32:(hl + 1) * 32, hl * AGP:hl * AGP + AGENT],
                in_=agentT[hf][hl * 32:(hl + 1) * 32, :])
            nc.vector.tensor_copy(
                out=b49[hl * 32:(hl + 1) * 32,
                        hl * AGENT:(hl + 1) * AGENT],
                in_=agentT[hf][hl * 32:(hl + 1) * 32, :])
        bd.append(b)
        bd49.append(b49)

    # prefetch next sample's input; same tags, so the DMA waits for this
    # sample's last xh reader and overlaps the rest of this sample
    xh_next = load_x(s + 1) if s + 1 < SPC else None

    # ---- dwc: diagonal matmuls, x-wrap columns fixed on DVE -------------
    DWall = big.tile([128, 2, N], BF16, name="dwall", tag="dwall")
    TAPS = [(0, 0)] + [(dy, dx) for dy in (-1, 0, 1) for dx in (-1, 0, 1)
                       if (dy, dx) != (0, 0)]
    for cti in range(2):
        v = VT[cti]
        for chn in range(NCH):
            ps = psA.tile([128, F], F32, name="ps_dw", tag="ps_main")
            lo = chn * F
            for k, (dy, dx) in enumerate(TAPS):
                t9 = (dy + 1) * 3 + (dx + 1)
                d = dy * W + dx
                a = max(0, -(lo + d))
                b_ = max(0, (lo + F + d) - N)
                nc.tensor.matmul(
                    ps[:, a:F - b_], lhsT=ct["DIAG"][:, cti, t9, :],
                    rhs=v[:, lo + d + a:lo + F + d - b_],
                    start=(k == 0), stop=(k == 8), skip_group_check=True)
            nc.scalar.copy(out=DWall[:, cti, lo:lo + F], in_=ps)
        # x-boundary columns: recompute exactly with strided DVE ops
        dwp = DWall[:, cti, :]
        for xb, dxs in ((0, (0, 1)), (W - 1, (-1, 0))):
            first = True
            for dy in (0, -1, 1):     # dy=0 first: full row range overwrite
                for dx in dxs:
                    t9 = (dy + 1) * 3 + (dx + 1)
                    rs, re = max(0, -dy), H - max(0, dy)
                    nr = re - rs
                    o_ap = bass.AP(tensor=dwp.tensor,
                                   offset=dwp.offset + rs * W + xb,
                                   ap=[dwp.ap[0], [W, nr]])
                    v_ap = bass.AP(tensor=v.tensor,
                                   offset=v.offset + (rs + dy) * W + xb + dx,
                                   ap=[v.ap[0], [W, nr]])
                    wcol = ct["W9"][:, cti, t9:t9 + 1]
                    if first:
                        nc.vector.tensor_scalar_mul(
                            out=o_ap, in0=v_ap, scalar1=wcol)
                        first = False
                    else:
                        nc.vector.scalar_tensor_tensor(
                            out=o_ap, in0=v_ap, scalar=wcol, in1=o_ap,
                            op0=mybir.AluOpType.mult, op1=mybir.AluOpType.add)

    # ---- stage 1: agent -> kv attention (token-major) -------------------
    ps_av = psAcc.tile([65, 4 * 2 * AGENT], F32, name="ps_av",
                       tag="ps_av")
    # clear has_written bits for the whole bank
    nc.tensor.matmul(ps_av[0:65, 0:128], lhsT=ct["zero1"][:, 0:65],
                     rhs=ct["ones1"], start=True, stop=False,
                     skip_group_check=True)
    # av8 matmuls are skewed one tile behind the score/exp pipeline so the
    # PE never waits on the exp of the tile it just scored.
    e1s = {}
    btps = {}
    for t in range(NT + 1):
        if t < NT:
            if t % 2 == 0:
                btp = work.tile([128, 2, 8 * AGENT], BF16, name="bt",
                                tag="bt")
                nc.sync.dma_start(out=btp, in_=ct["bias1_d"].ap()[t // 2])
                btps[t // 2] = btp
            btp = btps[t // 2]
            ps = psA.tile([128, 8 * AGENT], F32, name="ps_s1",
                          tag="ps_main")
            nc.tensor.matmul(ps, lhsT=ct["ident"], rhs=btp[:, t % 2, :],
                             start=True, stop=False, skip_group_check=True)
            for hf in range(2):
                nc.tensor.matmul(
                    ps[:, hf * 4 * AGENT:(hf + 1) * 4 * AGENT],
                    lhsT=KT[hf][:, t * 128:(t + 1) * 128],
                    rhs=bd49[hf], start=False, stop=(hf == 1),
                    skip_group_check=True)
            e1 = work.tile([128, 8 * AGENT], BF16, name="e1", tag="e1")
            nc.scalar.activation(out=e1, in_=ps, func=AF.Exp)
            e1s[t] = e1
        if t >= 1:
            e1 = e1s.pop(t - 1)
            for p in range(NPAIR):
                nc.tensor.matmul(
                    ps_av[0:65, p * 2 * AGENT:(p + 1) * 2 * AGENT],
                    lhsT=vtok5[:, t - 1, p, :],
                    rhs=e1[:, p * 2 * AGENT:(p + 1) * 2 * AGENT],
                    start=False, stop=(t - 1 == NT - 1),
                    skip_group_check=True)

    # ---- stage 2 (agent-major) + fused output, 2-chunk software pipeline
    a2n = big.tile([128, NPAIR, N], BF16, name="a2n", tag="a2n")
    WAV = small.tile([128, NPAIR, DIM], BF16, name="wav", tag="wav")

    scale48 = small.tile([128, NPAIR], F32, name="scale48", tag="rcp4")

    def s1_post_a():
        # colsum row -> SBUF, transpose per head into [49,8], reciprocal,
        # then assemble the 64-padded per-pair scale columns
        cs1 = small.tile([65, 8 * AGENT], F32, name="cs1", tag="cs1")
        nc.scalar.copy(out=cs1[64:65, :], in_=ps_av[64:65, :])
        ps8 = psB.tile([AGENT, HEADS], F32, name="ps_cs8", tag="ps_s2",
                       bufs=1)
        for h in range(HEADS):
            nc.tensor.transpose(
                ps8[:, h:h + 1],
                in_=cs1[64:65, h * AGENT:(h + 1) * AGENT],
                identity=ct["ones65"][64:65, :])
        rcp8 = small.tile([AGENT, HEADS], F32, name="rcp8", tag="cs1")
        nc.vector.reciprocal(out=rcp8, in_=ps8)
        nc.vector.memset(scale48, 1.0)
        for h in range(HEADS):
            p = h // 2
            ro = 0 if h % 2 == 0 else 64
            nc.scalar.copy(out=scale48[ro:ro + AGENT, p:p + 1],
                           in_=rcp8[:, h:h + 1])
        avbd = ct["avbd"]
        for p in range(NPAIR):
            nc.scalar.copy(
                out=avbd[0:32, p, 0:AGENT],
                in_=ps_av[0:32, p * 2 * AGENT:p * 2 * AGENT + AGENT])
            nc.scalar.copy(
                out=avbd[32:64, p, 64:64 + AGENT],
                in_=ps_av[32:64,
                          p * 2 * AGENT + AGENT:(p + 1) * 2 * AGENT])

    def s1_post_b():
        avbd = ct["avbd"]
        for p in range(NPAIR):
            psW = psB.tile([128, DIM], F32, name="ps_wav", tag="ps_out",
                           bufs=2)
            nc.tensor.matmul(psW, lhsT=avbd[:, p, :],
                             rhs=ct["WprojP"][:, p, :], start=True, stop=True)
            nc.scalar.activation(out=WAV[:, p, :], in_=psW, func=AF.Copy,
                                 scale=scale48[:, p:p + 1])

    def s2_scores(c):
        bt2 = work.tile([128, NPAIR, F], BF16, name="btT", tag="bt")
        nc.sync.dma_start(out=bt2, in_=ct["biasT2_d"].ap()[c])
        for p in range(NPAIR):
            ps = psA.tile([128, F], F32, name="ps_s2", tag="ps_main")
            nc.tensor.matmul(ps, lhsT=ct["ident"], rhs=bt2[:, p, :],
                             start=True, stop=False, skip_group_check=True)
            nc.tensor.matmul(
                ps, lhsT=bd[p // 2][:, (p % 2) * 128:(p % 2) * 128 + 128],
                rhs=QT[p // 2][:, c * F:(c + 1) * F],
                start=False, stop=True, skip_group_check=True)
            nc.scalar.activation(out=a2n[:, p, c * F:(c + 1) * F], in_=ps,
                                 func=AF.Exp)
        ps_s2 = psB.tile([8, F], F32, name="ps_den", tag="ps_s2", bufs=1)
        for p in range(NPAIR):
            nc.tensor.matmul(
                ps_s2, lhsT=ct["ind8"][:, p, :],
                rhs=a2n[:, p, c * F:(c + 1) * F],
                start=(p == 0), stop=(p == NPAIR - 1))
        # r2 = 1/s2 via exp(-ln(s2)) on the scalar engine (s2 in [46, 53])
        r2f = small.tile([8, F], F32, name="r2f", tag="r2f")
        nc.scalar.activation(out=r2f, in_=ps_s2, func=AF.Ln)
        r2c = small.tile([8, F], BF16, name="r2c", tag="r2c")
        nc.scalar.activation(out=r2c, in_=r2f, func=AF.Exp, scale=-1.0)
        return r2c

    def s2_norm(c, r2c):
        for p in range(NPAIR):
            ps_r2 = psB.tile([128, F], F32, name="ps_r2", tag="ps_r2",
                             bufs=1)
            nc.tensor.matmul(ps_r2, lhsT=ct["indp"][:, p, :], rhs=r2c,
                             start=True, stop=True)
            sl = a2n[:, p, c * F:(c + 1) * F]
            nc.vector.tensor_mul(out=sl, in0=sl, in1=ps_r2)

    def s2_out(c):
        for ot in range(2):
            ps_o = psB.tile([128, F], F32, name="ps_o", tag="ps_out", bufs=2)
            for p in range(NPAIR):
                nc.tensor.matmul(
                    ps_o, lhsT=WAV[:, p, ot * 128:(ot + 1) * 128],
                    rhs=a2n[:, p, c * F:(c + 1) * F],
                    start=(p == 0), stop=False, skip_group_check=True)
            for kt_ in range(2):
                nc.tensor.matmul(
                    ps_o,
                    lhsT=ct["WprojT"][:, kt_, ot * 128:(ot + 1) * 128],
                    rhs=DWall[:, kt_, c * F:(c + 1) * F],
                    start=False, stop=(kt_ == 1), skip_group_check=True)
            o = outp.tile([128, F], F16, name="o_st", tag="o_st")
            nc.vector.tensor_scalar_add(
                out=o, in0=ps_o, scalar1=ct["bproj2"][:, ot:ot + 1])
            nc.gpsimd.dma_start(
                out=out_d.ap()[s, ot * 128:(ot + 1) * 128,
                               c * F:(c + 1) * F],
                in_=o)

    r2cs = {}
    for c in range(NCH + 2):
        if c < NCH:
            r2cs[c] = s2_scores(c)
        if c == 0:
            s1_post_a()
        if c == 1:
            s1_post_b()
        if 1 <= c <= NCH:
            s2_norm(c - 1, r2cs.pop(c - 1))
        if c >= 2:
            s2_out(c - 2)
    return xh_next


def kernel(**inputs):
    x = np.asarray(inputs["x"], np.float32)
    host = build_host_constants(
        np.asarray(inputs["Wq"], np.float32),
        np.asarray(inputs["Wkv"], np.float32),
        np.asarray(inputs["Wproj"], np.float32),
        np.asarray(inputs["bproj"], np.float32),
        np.asarray(inputs["Wdwc"], np.float32),
        np.asarray(inputs["bdwc"], np.float32),
        np.asarray(inputs["an_bias"], np.float32),
        np.asarray(inputs["na_bias"], np.float32),
        np.asarray(inputs["ah_bias"], np.float32),
        np.asarray(inputs["aw_bias"], np.float32),
        np.asarray(inputs["ha_bias"], np.float32),
        np.asarray(inputs["wa_bias"], np.float32),
    )
    nc = build_nc()

    specs = CONST_SPECS + STREAM_SPECS
    np_dt = {mybir.dt.float32: np.float32, mybir.dt.float16: np.float16,
             mybir.dt.bfloat16: ml_dtypes.bfloat16}

    const_map = {name: np.asarray(host[name], np_dt[dt])
                 for name, dt, _ in specs}
    xs = x.reshape(B, DIM, N)
    in_maps = []
    for c in range(N_CORES):
        m = dict(const_map)
        m["x"] = np.asarray(xs[c * SPC:(c + 1) * SPC], ml_dtypes.bfloat16)
        in_maps.append(m)

    res = run_bass_kernel_spmd(nc, in_maps, core_ids=list(range(N_CORES)))
    out = np.concatenate([res.results[c]["out"] for c in range(N_CORES)],
                         axis=0)
    return out.astype(np.float32).reshape(B, DIM, H, W)
